# revision 25
# baseline (speedup 1.0000x reference)
"""Multi-head attention (no mask) Trainium2 kernel, SPMD over 8 NeuronCores.

Problem: x[2,2048,1024] @ wq/wk/wv[1024,1024] (+zero biases) -> 16-head
scaled-dot-product attention (softmax over full sequence, no causal mask),
output [2,2048,1024] fp32.

Sharding: batch x head-quad. Core i handles batch i//4 and heads
4*(i%4)..4*(i%4)+4 (256 output columns). Host concatenates heads and
stacks batches.

v2 design (ACT-exp is the hard floor: 16.8M exps/core ~= 109us busy):
  - fp16 datapath everywhere below the fp32 psum accumulators.
  - scores: row-paired (h0 rows 0-63 / h1 rows 64-127) 64-contraction
    matmuls into uniform 3-slot psum tiles (two alternating 6KB tags =
    the double buffer), exp'd by ACT in N=1536 batches - 86 instrs/rep.
  - AV: col-tiled pairs - h0 -> psy[0:64], h1 -> psy[64:128] - full
    128-key contraction, both heads concurrent, accumulated over 16 key
    chunks in one psum bank (psy, bufs=1).
  - softmax denominator: DVE accumulates exp'd slots into dacc[128,2h,512]
    (2-byte mode); per phase one col-tiled pair of 64-wide ones-matmuls
    broadcast-reduces dacc into psd[128,512] (the broadcast is free); DVE
    reciprocal + multiply normalizes psy; transpose-DMA + SWDGE-cast
    (fp16 -> fp32) write the output. psd shares its bank with the
    projection accumulator (tag "d").
  - PE FIFO discipline (the in-order queues are the whole game):
    * AVs of exp-group g are flushed after the scores of group g+1, so
      the PE never blocks ahead of ACT's score supply;
    * a phase's last dacc adds are emitted one iteration before its
      finalize can fire (AV trailing), keeping the denominator complete;
    * finalize DVE work (recip/mult) is emitted before the iteration's
      dacc adds so psy's release never queues behind an exp wait;
    * phase-opening AVs are held one iteration after a finalize so the
      single psy bank is already free when they issue.
  - V reaches [key, dim] layout via one transpose-DMA per head pair;
    x/w reach fp16 via SWDGE DRAM casts + transpose-DMA (x) / direct (w).
  - trickle: projections are split into half-proj items, at most one per
    group; the next rep's prologue (DMA + hp0 projections) rides the
    back half of the current rep.

Measured (slope of reps=17 vs 1, min-of-84-samples): ~118us/rep vs
207.7us baseline; rel err 1.0e-3.
"""

import os
import sys

import numpy as np

for _p in ("/opt/trn_rl_repo", "/root/.axon_site/_ro/trn_rl_repo"):
    if _p not in sys.path and os.path.isdir(_p):
        sys.path.append(_p)

from contextlib import ExitStack

import concourse.bass as bass
import concourse.tile as tile
from concourse import bacc, mybir
from concourse.bass_utils import run_bass_kernel_spmd

FP32 = mybir.dt.float32
FP16 = mybir.dt.float16
Exp = mybir.ActivationFunctionType.Exp
Add = mybir.AluOpType.add
Mult = mybir.AluOpType.mult

N_CORES = 8
B, S, D = 2, 2048, 1024
COLS = 256            # output columns per core = 4 heads x 64
HP = 2                # head pairs per core
HD = 64               # head dim
SCALE = 0.125         # 1 / sqrt(HD)
QCH = 512             # q chunk (psum free dim)
NKC = 16              # 128-key chunks
NQC = S // QCH        # 4
DT = D // 128         # 8 contraction tiles for projections
NPH = HP * NQC        # 8 phases
SLOTS_PER_PH = NKC * 2

_CACHED_NC = None


def build_nc(reps=1):
    nc = bacc.Bacc("TRN2", target_bir_lowering=False, debug=False,
                   num_devices=N_CORES)

    x = nc.dram_tensor("x", [S, D], FP32, kind="ExternalInput").ap()
    w_ap = {}
    b_ap = {}
    for p in ("q", "k", "v"):
        w_ap[p] = nc.dram_tensor(f"w{p}", [D, COLS], FP32,
                                 kind="ExternalInput").ap()
        b_ap[p] = nc.dram_tensor(f"b{p}", [COLS], FP32,
                                 kind="ExternalInput").ap()
    out = nc.dram_tensor("out", [S, COLS], FP32, kind="ExternalOutput").ap()

    with tile.TileContext(nc) as tc, ExitStack() as ctx:
        dram_pool = ctx.enter_context(tc.tile_pool(name="dram", bufs=2,
                                                   space="DRAM"))
        const_pool = ctx.enter_context(tc.tile_pool(name="const", bufs=1))
        w_pool = ctx.enter_context(tc.tile_pool(name="w", bufs=2))
        xt_pool = ctx.enter_context(tc.tile_pool(name="xt", bufs=2))
        qkv_pool = ctx.enter_context(tc.tile_pool(name="qkv", bufs=2))
        v64_pool = ctx.enter_context(tc.tile_pool(name="v64", bufs=2))
        att_pool = ctx.enter_context(tc.tile_pool(name="att", bufs=6))
        dacc_pool = ctx.enter_context(tc.tile_pool(name="dacc", bufs=2))
        fin_pool = ctx.enter_context(tc.tile_pool(name="fin", bufs=2))
        ps_s = ctx.enter_context(tc.tile_pool(name="pss", bufs=1,
                                              space="PSUM"))
        ps_y = ctx.enter_context(tc.tile_pool(name="psy", bufs=1,
                                              space="PSUM"))

        ones = const_pool.tile([128, HD], FP16, tag="ones")
        nc.vector.memset(ones[:], 1.0)

        # ---- prologue emitters -------------------------------------------
        def emit_w_dma(st, p):
            # fp32 DRAM -> fp16 DRAM via SWDGE cast, then straight to SBUF
            w16 = dram_pool.tile([D, COLS], FP16, tag=f"w16{p}",
                                 name=f"w16{p}")
            nc.gpsimd.dma_start(out=w16.rearrange("a b -> (a b)"),
                                in_=w_ap[p].rearrange("a b -> (a b)"))
            wt = w_pool.tile([128, DT, COLS], FP16, tag=f"w{p}",
                             name=f"wt{p}")
            nc.sync.dma_start(out=wt[:],
                              in_=w16.rearrange("(t p) c -> p t c", p=128))
            st["w", p] = wt
            bt = w_pool.tile([128, HP], FP32, tag=f"b{p}", name=f"bt{p}")
            nc.sync.dma_start(out=bt[:],
                              in_=b_ap[p].rearrange("(hp c) -> c hp", c=128))
            st["b", p] = bt

        def emit_xcast(st, sc):
            if "x16" not in st:
                st["x16"] = dram_pool.tile([S, D], FP16, tag="x16",
                                           name="x16")
            nc.gpsimd.dma_start(
                out=st["x16"][sc * QCH:(sc + 1) * QCH, :].rearrange(
                    "a b -> (a b)"),
                in_=x[sc * QCH:(sc + 1) * QCH, :].rearrange("a b -> (a b)"))

        def emit_xt(st, t):
            if "xt" not in st:
                st["xt"] = xt_pool.tile([128, DT, S], FP16, tag="xt",
                                        name="xt")
            nc.sync.dma_start(out=st["xt"][:, t, :],
                              in_=st["x16"][:, t * 128:(t + 1) * 128],
                              transpose=True)

        # ---- projections --------------------------------------------------
        def emit_proj_mms(st, hp, p, sc, t0, t1):
            key = (hp, p, "ps", sc)
            if key not in st:
                # shares the psd bank (tag "d"): projections and the
                # per-phase denominator reduce alternate through it
                st[key] = ps_y.tile([128, QCH], FP32, tag="d", bufs=1,
                                    name="pspj")
            ps = st[key]
            xt = st["xt"]
            wt = st["w", p]
            for t in range(t0, t1):
                nc.tensor.matmul(
                    ps[:], lhsT=wt[:, t, hp * 128:(hp + 1) * 128],
                    rhs=xt[:, t, sc * QCH:(sc + 1) * QCH],
                    start=(t == 0), stop=(t == DT - 1))

        def emit_proj_copy(st, hp, p, sc):
            if (hp, p) not in st:
                st[hp, p] = qkv_pool.tile([128, S], FP16, tag=f"{p}T{hp}",
                                          name=f"{p}T{hp}")
            ps = st.pop((hp, p, "ps", sc))
            nc.vector.tensor_scalar_add(
                st[hp, p][:, sc * QCH:(sc + 1) * QCH], ps[:],
                st["b", p][:, hp:hp + 1])

        def emit_v64(st, hp):
            # vT[hp] [128(2h x 64d), 2048k] -> v64 [128k, 16K, 128d2]
            v64 = v64_pool.tile([128, NKC, 128], FP16, tag=f"v64{hp}",
                                name=f"v64{hp}")
            nc.sync.dma_start(out=v64[:], in_=st[hp, "v"][:], transpose=True)
            st[hp, "v64"] = v64

        def proj_items(st, hp, p, sc):
            # (cost_ns, thunk) halves: a proj spans two adjacent groups so
            # the shared psum bank frees quickly
            return [
                (900, lambda: emit_proj_mms(st, hp, p, sc, 0, 4)),
                (950, lambda: (emit_proj_mms(st, hp, p, sc, 4, 8),
                               emit_proj_copy(st, hp, p, sc))),
            ]

        def prologue_dma_items(st):
            items = []
            for p in ("q", "k", "v"):
                items.append((100, lambda p=p: emit_w_dma(st, p)))
            for sc in range(NQC):
                items.append((100, lambda sc=sc: emit_xcast(st, sc)))
            for t in range(DT):
                items.append((100, lambda t=t: emit_xt(st, t)))
            return items

        def prologue_pe_items(st):
            # everything the first phase (hp0, qc0) needs: k all-sc (keys
            # sweep the full sequence), q sc0, v all-sc + v64.
            items = []
            for sc in range(NQC):
                items += proj_items(st, 0, "k", sc)
            items += proj_items(st, 0, "q", 0)
            for sc in range(NQC):
                items += proj_items(st, 0, "v", sc)
            items.append((100, lambda: emit_v64(st, 0)))
            return items

        def own_trickle_items(st):
            # ordered so each dependency lands a few groups early: q(qc)
            # before phase (0, qc); all hp1 k/v + v64 before slot 128.
            items = proj_items(st, 0, "q", 1)
            for sc in range(2):
                items += proj_items(st, 1, "k", sc)
                items += proj_items(st, 1, "v", sc)
            items += proj_items(st, 0, "q", 2)
            for sc in range(2, NQC):
                items += proj_items(st, 1, "k", sc)
                items += proj_items(st, 1, "v", sc)
            items.append((100, lambda: emit_v64(st, 1)))
            items += proj_items(st, 0, "q", 3)
            for sc in range(NQC):
                items += proj_items(st, 1, "q", sc)
            return items

        # ---- finalize -----------------------------------------------------
        def emit_finalize(st, hp, qc, psy, dacc):
            psd = ps_y.tile([128, QCH], FP32, tag="d", bufs=1, name="psd")
            for h in range(2):
                nc.tensor.matmul(
                    psd[h * HD:(h + 1) * HD, :], lhsT=ones[:],
                    rhs=dacc[:, h, :], start=True, stop=True,
                    tile_position=(0, h * HD))
            rp = fin_pool.tile([128, QCH], FP16, tag="rp", name="rp")
            y16 = fin_pool.tile([128, QCH], FP16, tag="y16", name="y16")
            with nc.allow_low_precision(reason="fp16 softmax normalize"):
                nc.vector.reciprocal(rp[:], psd[:])
                nc.vector.tensor_tensor(y16[:], psy[:], rp[:], Mult)
            yo = fin_pool.tile([128, NQC, 128], FP16, tag="yo", name="yo")
            nc.sync.dma_start(out=yo[:], in_=y16[:], transpose=True)
            nc.gpsimd.dma_start(
                out=out[qc * QCH:(qc + 1) * QCH,
                        hp * 128:(hp + 1) * 128].rearrange(
                    "(j p) c -> p j c", p=128),
                in_=yo[:])

        # ---- the attention stream ----------------------------------------
        def run_rep(st, own_items, late_items, late_start):
            own_items = list(own_items)[::-1]
            late_items = list(late_items)[::-1]

            slots = [(hp, qc, K, h)
                     for hp in range(HP) for qc in range(NQC)
                     for K in range(NKC) for h in range(2)]
            ns = len(slots)
            groups = []
            i = 0
            while i < ns:
                sz = min(3, ns - i)
                groups.append((i, sz))
                i += sz

            att_of = {}          # global slot idx -> (att_tile, idx_in_tile)
            fin_flag = [False]   # a finalize was emitted this iteration
            av_ready = []        # global slot idxs whose att is available
            av_deferred = []     # held back to keep h-pairs adjacent
            phase_av_count = {}
            phase_psy = {}
            phase_dacc = {}
            phase_first = {}     # (ph, h) -> True until first dacc write

            def get_dacc(ph):
                if ph not in phase_dacc:
                    phase_dacc[ph] = dacc_pool.tile([128, 2, QCH], FP16,
                                                    tag="dacc", name="dacc")
                    phase_first[ph, 0] = True
                    phase_first[ph, 1] = True
                return phase_dacc[ph]

            def get_psy(ph):
                # only called from emit_av: phase ph's first AV is emitted
                # strictly after the previous phase's finalize, so the
                # bufs=1 rotation sees all prior readers already emitted.
                if ph not in phase_psy:
                    phase_psy[ph] = ps_y.tile([128, QCH], FP32, tag="y",
                                              bufs=1, name="psy")
                    phase_av_count[ph] = 0
                return phase_psy[ph]

            def emit_av(si):
                hp, qc, K, h = slots[si]
                ph = hp * NQC + qc
                psy = get_psy(ph)
                at, idx = att_of.pop(si)
                nc.tensor.matmul(
                    psy[h * HD:(h + 1) * HD, :],
                    lhsT=st[hp, "v64"][:, K, h * HD:(h + 1) * HD],
                    rhs=at[:, idx, :],
                    start=(K == 0), stop=(K == NKC - 1),
                    tile_position=(0, h * HD))
                phase_av_count[ph] += 1
                if phase_av_count[ph] == SLOTS_PER_PH:
                    emit_finalize(st, hp, qc, phase_psy.pop(ph),
                                  phase_dacc.pop(ph))
                    fin_flag[0] = True

            def flush_avs(final=False):
                pend = av_deferred + av_ready
                av_ready.clear()
                av_deferred.clear()
                while pend:
                    # a phase-opening AV reuses the single psy bank; hold it
                    # one iteration after a finalize so the PE FIFO doesn't
                    # block on the DVE mult that frees the bank
                    if (not final and fin_flag[0]
                            and slots[pend[0]][2:] == (0, 0)):
                        break
                    if len(pend) == 1 and not final:
                        break  # keep h-pairs adjacent across batches
                    emit_av(pend.pop(0))
                av_deferred.extend(pend)

            def emit_dacc(gstart, gsize, at):
                # batch adds into dacc[:, h, :]; h == global idx parity.
                i = 0
                while i < gsize:
                    si = gstart + i
                    hp, qc, K, h = slots[si]
                    ph = hp * NQC + qc
                    dacc = get_dacc(ph)
                    # run of 2 with h==0 first, staying inside the phase
                    run2 = (h == 0 and i + 1 < gsize
                            and slots[si + 1][:2] == (hp, qc))
                    with nc.allow_low_precision(reason="fp16 denom accum"):
                        if run2:
                            dst = dacc[:, 0:2, :]
                            src = at[:, i:i + 2, :]
                            if phase_first[ph, 0] or phase_first[ph, 1]:
                                nc.vector.tensor_copy(dst, src)
                            else:
                                nc.vector.tensor_tensor(dst, dst, src, Add)
                            phase_first[ph, 0] = False
                            phase_first[ph, 1] = False
                            i += 2
                        else:
                            dst = dacc[:, h, :]
                            src = at[:, i, :]
                            if phase_first[ph, h]:
                                nc.vector.tensor_copy(dst, src)
                                phase_first[ph, h] = False
                            else:
                                nc.vector.tensor_tensor(dst, dst, src, Add)
                            i += 1

            for gi, (gstart, gsize) in enumerate(groups):
                tag = "sA" if gi % 2 == 0 else "sB"
                pss = ps_s.tile([128, 3, QCH], FP32, tag=tag, bufs=1,
                                name="pss")
                for i in range(gsize):
                    hp, qc, K, h = slots[gstart + i]
                    nc.tensor.matmul(
                        pss[:, i, :],
                        lhsT=st[hp, "k"][h * HD:(h + 1) * HD,
                                         K * 128:(K + 1) * 128],
                        rhs=st[hp, "q"][h * HD:(h + 1) * HD,
                                        qc * QCH:(qc + 1) * QCH],
                        start=True, stop=True)
                at = att_pool.tile([128, 3, QCH], FP16, tag="att",
                                   name="att")
                with nc.allow_low_precision(reason="fp16 attention"):
                    nc.scalar.activation(at[:, 0:gsize, :],
                                         pss[:, 0:gsize, :], Exp,
                                         scale=SCALE)
                for i in range(gsize):
                    att_of[gstart + i] = (at, i)

                # trickle (always-ready PE work) before the AV batch:
                # at most ONE item per group keeps PE per-period smooth
                if own_items:
                    own_items.pop()[1]()
                elif gi >= late_start and late_items:
                    late_items.pop()[1]()

                # AVs trail by one group (extend below), so a finalize
                # fired here reads a dacc completed last iteration; putting
                # the flush before emit_dacc keeps the finalize's DVE ops
                # (recip/mult -> psy release) ahead of dacc(g), which waits
                # on exp(g) in the DVE FIFO.
                fin_flag[0] = False
                flush_avs()
                emit_dacc(gstart, gsize, at)
                av_ready.extend(range(gstart, gstart + gsize))

            flush_avs(final=True)
            while own_items:
                own_items.pop()[1]()
            while late_items:
                late_items.pop()[1]()

        # ---- rep loop -----------------------------------------------------
        sts = [{} for _ in range(reps)]
        for _, it in prologue_dma_items(sts[0]):
            it()
        for _, it in prologue_pe_items(sts[0]):
            it()
        for r in range(reps):
            late = []
            if r + 1 < reps:
                late = prologue_dma_items(sts[r + 1]) \
                    + prologue_pe_items(sts[r + 1])
            run_rep(sts[r], own_trickle_items(sts[r]), late,
                    late_start=44)

    nc.compile()
    return nc


def get_nc():
    global _CACHED_NC
    if _CACHED_NC is None:
        _CACHED_NC = build_nc()
    return _CACHED_NC


def make_in_maps(x, wq, bq, wk, bk, wv, bv):
    in_maps = []
    for i in range(N_CORES):
        b = i // 4
        c0 = (i % 4) * COLS
        in_maps.append({
            "x": np.ascontiguousarray(x[b], dtype=np.float32),
            "wq": np.ascontiguousarray(wq[:, c0:c0 + COLS], dtype=np.float32),
            "wk": np.ascontiguousarray(wk[:, c0:c0 + COLS], dtype=np.float32),
            "wv": np.ascontiguousarray(wv[:, c0:c0 + COLS], dtype=np.float32),
            "bq": np.ascontiguousarray(bq[c0:c0 + COLS], dtype=np.float32),
            "bk": np.ascontiguousarray(bk[c0:c0 + COLS], dtype=np.float32),
            "bv": np.ascontiguousarray(bv[c0:c0 + COLS], dtype=np.float32),
        })
    return in_maps


def assemble(res, inputs=None):
    batches = []
    for b in range(B):
        parts = [res.results[b * 4 + q]["out"] for q in range(4)]
        batches.append(np.concatenate(parts, axis=1))
    return np.stack(batches).astype(np.float32)


def kernel(x, wq, bq, wk, bk, wv, bv):
    nc = get_nc()
    in_maps = make_in_maps(x, wq, bq, wk, bk, wv, bv)
    res = run_bass_kernel_spmd(nc, in_maps, list(range(N_CORES)))
    out = assemble(res)
    kernel.last_results = res
    return out
